# revision 9
# baseline (speedup 1.0000x reference)
"""Bahdanau attention kernel for Trainium2 (Bass/Tile), 8 NeuronCores.

Problem (per batch element b):
    q_proj = query[b] @ w1.T          # (LQ, H)
    k_proj = key[b]   @ w2.T          # (LK, H)
    score[q, k] = sum_h v[h] * tanh(q_proj[q, h] + k_proj[k, h])
    attn = softmax(score, axis=-1)    # output 1
    ctx  = attn @ value[b]            # output 2

Sharding: data-parallel over batch B=8 across the 8 cores (no collectives).
Host prep: query/key/w1/w2/v are passed pre-transposed so the device never
transposes inputs (contraction dim must live on SBUF partitions for the PE).

Per-core device schedule:
  - qpT/kpT = [h=128, l=512] projections via float32r matmuls.
  - Main loop over 64 subtiles of 8 queries:
      DVE tensor_scalar_add broadcasts qpT[:, q] over kpT  -> s[h, 8, 512]
      ACT tanh over the whole [128, 4096] tile             -> t
      8 PE matmuls, vT [h,1] stationary, t[:, j, :] moving -> score rows (PSUM)
  - Per 128-query block: row max (negated) -> exp(bias=-max, accum_out=sums)
    -> reciprocal -> scale -> attn out; PE-transpose of p -> 4 matmuls with
    value -> scale -> ctx out.
"""

import numpy as np

import concourse.bass as bass
import concourse.mybir as mybir
import concourse.tile as tile
from concourse import bacc
from concourse.bass_utils import run_bass_kernel_spmd
from concourse.masks import make_identity

F32 = mybir.dt.float32
F32R = mybir.dt.float32r

B = 8
L = 512          # LQ == LK
D = 512          # DQ == DK == DV
H = 128
P = 128          # SBUF partitions
NDB = D // P     # 4 d-blocks
NQB = L // P     # 4 query blocks
QSUB = 8         # queries per tanh subtile
NSUB = P // QSUB # 16 subtiles per query block

_CACHED_NC = None


def _build_nc():
    nc = bacc.Bacc("TRN2", target_bir_lowering=False, debug=False)

    qT = nc.dram_tensor("qT", [D, L], F32, kind="ExternalInput")      # query[b].T
    kT = nc.dram_tensor("kT", [D, L], F32, kind="ExternalInput")      # key[b].T
    val = nc.dram_tensor("val", [L, D], F32, kind="ExternalInput")    # value[b]
    w1T = nc.dram_tensor("w1T", [D, H], F32, kind="ExternalInput")    # w1.T
    w2T = nc.dram_tensor("w2T", [D, H], F32, kind="ExternalInput")    # w2.T
    # vwin[h, c] = v[h] if c == P else 0.  lhsT = vwin[:, P-r : 2P-r] puts v
    # in stationary column r, so matmul r adds score row r into the PSUM tile
    # (and +0 into every other row).
    vwin = nc.dram_tensor("vwin", [H, 2 * P], F32, kind="ExternalInput")
    attn = nc.dram_tensor("attn", [L, L], F32, kind="ExternalOutput")
    ctxo = nc.dram_tensor("ctx", [L, L], F32, kind="ExternalOutput")

    with tile.TileContext(nc) as tc:
        with (
            tc.tile_pool(name="const", bufs=1) as const,
            tc.tile_pool(name="s", bufs=2) as s_pool,
            tc.tile_pool(name="t", bufs=2) as t_pool,
            tc.tile_pool(name="p", bufs=2) as p_pool,
            tc.tile_pool(name="pt", bufs=8) as pt_pool,
            tc.tile_pool(name="outs", bufs=4) as out_pool,
            tc.tile_pool(name="stat", bufs=12) as stat_pool,
            tc.tile_pool(name="proj_ps", bufs=1, space="PSUM") as proj_ps_pool,
            tc.tile_pool(name="score_ps", bufs=2, space="PSUM") as score_ps_pool,
            tc.tile_pool(name="ctx_ps", bufs=2, space="PSUM") as ctx_ps_pool,
            tc.tile_pool(name="tp_ps", bufs=2, space="PSUM") as tp_ps_pool,
        ):
            # ---------------- prologue: loads ----------------
            ident = const.tile([P, P], F32)
            make_identity(nc, ident[:])

            vwin_f32 = const.tile([H, 2 * P], F32)
            nc.sync.dma_start(out=vwin_f32[:], in_=vwin[:, :])
            # FP32r matmul operands must be produced rounded-to-f32r by a
            # compute engine (walrus birverifier rule) — DVE-cast once.
            vwin_sb = const.tile([H, 2 * P], F32R)
            nc.vector.tensor_copy(vwin_sb[:], vwin_f32[:])

            val_sb = const.tile([P, NQB, D], F32)
            nc.sync.dma_start(
                out=val_sb[:], in_=val[:, :].rearrange("(kb p) d -> p kb d", p=P)
            )

            qT_sb = const.tile([P, NDB, L], F32)
            nc.sync.dma_start(
                out=qT_sb[:], in_=qT[:, :].rearrange("(db p) l -> p db l", p=P)
            )
            kT_sb = const.tile([P, NDB, L], F32)
            nc.sync.dma_start(
                out=kT_sb[:], in_=kT[:, :].rearrange("(db p) l -> p db l", p=P)
            )
            w1T_sb = const.tile([P, NDB, H], F32)
            nc.sync.dma_start(
                out=w1T_sb[:], in_=w1T[:, :].rearrange("(db p) h -> p db h", p=P)
            )
            w2T_sb = const.tile([P, NDB, H], F32)
            nc.sync.dma_start(
                out=w2T_sb[:], in_=w2T[:, :].rearrange("(db p) h -> p db h", p=P)
            )

            # ---------------- projections: qpT/kpT [h, l] ----------------
            qpT = const.tile([H, L], F32)
            kpT = const.tile([H, L], F32)
            for dst, wt, xt in ((qpT, w1T_sb, qT_sb), (kpT, w2T_sb, kT_sb)):
                ps = proj_ps_pool.tile([H, L], F32)
                for db in range(NDB):
                    nc.tensor.matmul(
                        ps[:],
                        wt[:, db, :],
                        xt[:, db, :],
                        start=(db == 0),
                        stop=(db == NDB - 1),
                    )
                nc.vector.tensor_copy(dst[:], ps[:])

            # ---------------- main loop ----------------
            for qb in range(NQB):
                score_ps = score_ps_pool.tile([P, L], F32)
                for sub in range(NSUB):
                    s_t = s_pool.tile([P, QSUB, L], F32)
                    for j in range(QSUB):
                        q = qb * P + sub * QSUB + j
                        nc.vector.tensor_scalar_add(
                            s_t[:, j, :], kpT[:], qpT[:, q : q + 1]
                        )
                    t_t = t_pool.tile([P, QSUB, L], F32R)
                    nc.scalar.activation(
                        t_t[:], s_t[:], mybir.ActivationFunctionType.Tanh
                    )
                    for j in range(QSUB):
                        row = sub * QSUB + j
                        nc.tensor.matmul(
                            score_ps[:],
                            vwin_sb[:, P - row : 2 * P - row],
                            t_t[:, j, :],
                            start=(row == 0),
                            stop=(row == P - 1),
                        )

                # softmax over k for the 128 q-rows of this block
                neg_max = stat_pool.tile([P, 1], F32)
                nc.vector.reduce_max(
                    neg_max[:], score_ps[:], axis=mybir.AxisListType.X, negate=True
                )
                p_t = p_pool.tile([P, L], F32)
                sums = stat_pool.tile([P, 1], F32)
                nc.scalar.activation(
                    p_t[:],
                    score_ps[:],
                    mybir.ActivationFunctionType.Exp,
                    bias=neg_max[:],
                    accum_out=sums[:],
                )
                inv = stat_pool.tile([P, 1], F32)
                nc.vector.reciprocal(inv[:], sums[:])

                attn_t = out_pool.tile([P, L], F32)
                nc.vector.tensor_scalar_mul(attn_t[:], p_t[:], inv[:])
                nc.sync.dma_start(
                    out=attn[qb * P : (qb + 1) * P, :], in_=attn_t[:]
                )

                # context: ctx[qb] = (p @ value) * inv
                pT_sbs = []
                for kb in range(NQB):
                    tp = tp_ps_pool.tile([P, P], F32)
                    nc.tensor.transpose(
                        tp[:], p_t[:, kb * P : (kb + 1) * P], ident[:]
                    )
                    pT_sb = pt_pool.tile([P, P], F32)
                    nc.vector.tensor_copy(pT_sb[:], tp[:])
                    pT_sbs.append(pT_sb)
                ctx_ps = ctx_ps_pool.tile([P, D], F32)
                for kb in range(NQB):
                    nc.tensor.matmul(
                        ctx_ps[:],
                        pT_sbs[kb][:],
                        val_sb[:, kb, :],
                        start=(kb == 0),
                        stop=(kb == NQB - 1),
                    )
                ctx_t = out_pool.tile([P, D], F32)
                nc.vector.tensor_scalar_mul(ctx_t[:], ctx_ps[:], inv[:])
                nc.sync.dma_start(
                    out=ctxo[qb * P : (qb + 1) * P, :], in_=ctx_t[:]
                )

    nc.compile()
    return nc


def _get_nc():
    global _CACHED_NC
    if _CACHED_NC is None:
        _CACHED_NC = _build_nc()
    return _CACHED_NC


def _in_maps(query, key, value, w1, w2, v):
    f = np.float32
    w1T = np.ascontiguousarray(np.asarray(w1, dtype=f).T)
    w2T = np.ascontiguousarray(np.asarray(w2, dtype=f).T)
    vwin = np.zeros((H, 2 * P), dtype=f)
    vwin[:, P] = np.asarray(v, dtype=f)[0]
    maps = []
    for b in range(B):
        maps.append(
            {
                "qT": np.ascontiguousarray(np.asarray(query[b], dtype=f).T),
                "kT": np.ascontiguousarray(np.asarray(key[b], dtype=f).T),
                "val": np.ascontiguousarray(np.asarray(value[b], dtype=f)),
                "w1T": w1T,
                "w2T": w2T,
                "vwin": vwin,
            }
        )
    return maps


def run(query, key, value, w1, w2, v, trace=False, **spmd_kwargs):
    nc = _get_nc()
    res = run_bass_kernel_spmd(
        nc,
        _in_maps(query, key, value, w1, w2, v),
        list(range(B)),
        trace=trace,
        **spmd_kwargs,
    )
    attn = np.stack([res.results[b]["attn"] for b in range(B)])
    ctx = np.stack([res.results[b]["ctx"] for b in range(B)])
    return (attn, ctx), res


def kernel(query, key, value, w1, w2, v):
    (attn, ctx), _ = run(query, key, value, w1, w2, v, trace=False)
    return (attn, ctx)
